# revision 32
# baseline (speedup 1.0000x reference)
"""Trainium2 Bass kernel for nn_Cheb_35888746725726 (ChebConv K=3 GNN, N=50000,
E=800000, F=H=96, lambda_max=2 -> diag term is 0).

v2 strategy (8 NeuronCores, node/graph-parallel):
 - Host: Chebyshev edge norm, capacity-bounded LPT of nodes into 392 dst-tiles
   of 128 (8 cores x 49), per-edge (pair-idx, dst-slot|parity, weight) planes.
 - Device tables are NODE-major DRAM [25088 pair-rows, 256] bf16 (two 128-col
   padded node rows per 512B row).  One bulk `dma_gather` per ~1024 edges
   fetches pair-rows straight into node-major SBUF tiles [128 edges, 256]; the
   wrong member of each pair is cancelled by a zero weight in the one-hot
   scatter matrices (host bakes parity), so no select/transpose is needed.
 - Scatter: per dst-tile, 2*TE2 accumulating PE matmuls (even/odd candidate
   slices x weighted one-hot [128,128]) -> PSUM [96,128] -> feature-major fm.
 - Tables for the next hop are written by 49 PE transposes of the fm shard
   (staged, For_i) + one strided DMA; AllGather (8 cores) rebuilds the full
   pair-row table between dependent props.
 - Dense 96x96 matmuls run feature-major with host-folded Chebyshev weights:
   out = Tx0 @ (W0-W2) + Tx1 @ W1 + (L@Tx1) @ (2*W2).
"""
import numpy as np
import ml_dtypes

import jax

try:
    jax.config.update("jax_compilation_cache_dir", "/tmp/jax_comp_cache")
    jax.config.update("jax_persistent_cache_min_compile_time_secs", 0.0)
    jax.config.update("jax_persistent_cache_min_entry_size_bytes", 0)
except Exception:
    pass

import concourse.bass as bass
import concourse.bacc as bacc
import concourse.mybir as mybir
import concourse.tile as tile
from concourse.bass import ds
from concourse.bass_utils import run_bass_kernel_spmd
from concourse.masks import make_identity

# ---- problem constants (hardcoded per the harness contract) ----
N = 50000
E = 800000
F = 96
C = 8                    # cores
NP_PAD = 50176           # 8 * 6272
SHARD = NP_PAD // C      # 6272
DT = 49                  # dst tiles per core
DTW = 128                # dst tile width (nodes)
TE2 = 17                 # 128-edge tiles per dst tile (capacity 2176 edges)
ECOLS = DT * TE2         # 833 per-edge plane columns
NPAIR = NP_PAD // 2      # 25088 pair rows
PAIRW = 256              # elements per pair row (2 x 128-col padded nodes)
IDXT = TE2 * DTW // 16   # 136 idx cols per dst tile
IDXCOLS = DT * IDXT      # 6664
P = 128
NCHUNK = 512             # dense matmul node-chunk

# blob column offsets (bf16 [128, CB]); narrow dtypes bitcast into bf16 cols.
# x is 10-bit (hi int8 + packed 2-bit nibbles; scale folds into layer-1
# weights); idx16 hides in rows 96:128 of the x cols. Edge |w| is sqrt-coded
# u8 (device squares it; scale/sign fold into the dense weights).
XC = SHARD // 2                     # 3136 bf16 cols holding int8 x-hi
XLC = SHARD // 8                    # 784 bf16 cols holding 2-bit x-lo quads
SPC = (ECOLS + 1) // 2              # 417
OFF_XL = XC                         # [0:96, XLC] x-lo u8 (bitcast)
OFF_W = XC + XLC                    # [128, SPC] edge sqrt|w| u8 (bitcast)
OFF_SP = OFF_W + SPC                # [128, SPC] slot|parity u8 (bitcast)
OFF_WM = OFF_SP + SPC               # [0:96, 6F+2] folded dense weights
OFF_B = OFF_WM + 6 * F + 2          # [0:96, 6] b1,b2,blin f32 bitcast
CB = OFF_B + 6
IDX1 = XC + XLC                     # idx strip 1 cols (rows 96:112)
IDX2 = IDXCOLS - IDX1               # strip 2 cols (rows 112:128)
assert IDX2 <= IDX1
assert OFF_B % 2 == 0 and CB % 2 == 0

BF = ml_dtypes.bfloat16

_compiled = None


# --------------------------------------------------------------------------
# host-side preprocessing
# --------------------------------------------------------------------------
def _preprocess(x, edge_index, edge_weight):
    src = np.asarray(edge_index[0]).astype(np.int64)
    dst = np.asarray(edge_index[1]).astype(np.int64)
    w = np.asarray(edge_weight).astype(np.float32)

    deg = np.zeros(N, np.float32)
    np.add.at(deg, src, w)
    dis = np.where(deg > 0, 1.0 / np.sqrt(np.maximum(deg, 1e-30)), 0.0).astype(np.float32)
    norm_w = (-dis[src] * w * dis[dst]).astype(np.float32)

    # capacity-bounded LPT: nodes -> 392 tiles of 128, indeg sum <= TE2*128
    indeg = np.bincount(dst, minlength=N).astype(np.int64)
    n_tiles = C * DT
    cap = TE2 * DTW
    order = np.argsort(-indeg, kind="stable")
    import heapq
    heap = [(0, 0, t) for t in range(n_tiles)]
    heapq.heapify(heap)
    tile_assign = np.empty(N, np.int64)
    spill = []
    for n in order:
        placed = False
        while heap:
            load, cnt, t = heapq.heappop(heap)
            if cnt < DTW and load + indeg[n] <= cap:
                tile_assign[n] = t
                heapq.heappush(heap, (load + indeg[n], cnt + 1, t))
                placed = True
                break
            if cnt < DTW:
                spill.append((load, cnt, t))
            # full tiles drop out
        for it in spill:
            heapq.heappush(heap, it)
        spill.clear()
        assert placed, "LPT infeasible: raise TE2"

    order2 = np.argsort(tile_assign, kind="stable")
    slot_in_tile = np.empty(N, np.int64)
    counts = np.bincount(tile_assign, minlength=n_tiles)
    starts = np.concatenate([[0], np.cumsum(counts)[:-1]])
    slot_in_tile[order2] = np.arange(N) - np.repeat(starts, counts)
    new_id = tile_assign * DTW + slot_in_tile

    src_n = new_id[src]
    dst_n = new_id[dst]

    # bucket edges into (core, edge-slot) by destination tile
    o = np.argsort(dst_n, kind="stable")
    es, ed, ew = src_n[o], dst_n[o], norm_w[o]
    gtile = ed // DTW
    tstart = np.searchsorted(gtile, np.arange(n_tiles))
    r = np.arange(E) - tstart[gtile]                  # rank within dst tile
    assert r.max() < TE2 * P, f"tile overflow: {r.max() + 1}"
    core = gtile // DT
    dtile = gtile % DT
    lane = r % P
    etile = r // P                                    # 0..TE2-1
    col = dtile * TE2 + etile

    idxp = np.zeros((C, P, ECOLS), np.int16)          # pair idx per edge slot
    wp = np.zeros((C, P, ECOLS), np.float32)
    sp = np.zeros((C, P, ECOLS), np.uint8)
    idxp[core, lane, col] = (es // 2).astype(np.int16)
    wp[core, lane, col] = ew
    sp[core, lane, col] = (ed - gtile * DTW).astype(np.uint8) | ((es % 2) << 7).astype(np.uint8)

    return new_id, idxp, wp, sp


_pre_cache = {}


def _preprocess_cached(x, edge_index, edge_weight):
    import hashlib
    ei = np.ascontiguousarray(edge_index)
    ew = np.ascontiguousarray(edge_weight)
    h = hashlib.blake2b(ei.tobytes(), digest_size=16)
    h.update(ew.tobytes())
    key = h.hexdigest()
    if key not in _pre_cache:
        _pre_cache.clear()
        _pre_cache[key] = _preprocess(x, edge_index, edge_weight)
    return _pre_cache[key]


# --------------------------------------------------------------------------
# bass kernel builder
# --------------------------------------------------------------------------
def _build_kernel(cfg=()):
    cfg = frozenset(cfg)
    dt = mybir.dt
    nc = bacc.Bacc("TRN2", target_bir_lowering=False, debug=False, num_devices=C)

    blob_d = nc.dram_tensor("blob", [P, CB], dt.bfloat16, kind="ExternalInput")
    out_d = nc.dram_tensor("out", [2, SHARD], dt.bfloat16, kind="ExternalOutput")

    rg = [list(range(C))]
    local_ag = "noag" in cfg
    nrep = 4 if "rep4" in cfg else 1

    with tile.TileContext(nc) as tc:
        with (
            tc.tile_pool(name="res", bufs=1) as res,
            tc.tile_pool(name="mpool", bufs=6) as mpool,      # gather dests
            tc.tile_pool(name="spool", bufs=3) as spool,      # small staging
            tc.tile_pool(name="opool", bufs=2) as opool,      # one-hot planes
            tc.tile_pool(name="pscat", bufs=2, space="PSUM") as pscat,
            tc.tile_pool(name="ptr", bufs=2, space="PSUM") as ptr,
            tc.tile_pool(name="pdense", bufs=2, space="PSUM") as pdense,
            tc.tile_pool(name="dram", bufs=1, space="DRAM") as dram,
        ):
            # ---------- resident loads ----------
            fm = {
                "tx0": res.tile([F, SHARD], dt.bfloat16, name="fm_tx0"),
                "t1": res.tile([F, SHARD], dt.bfloat16, name="fm_t1"),
                "s2": res.tile([F, SHARD], dt.bfloat16, name="fm_s2"),
                "h": res.tile([F, SHARD], dt.bfloat16, name="fm_h"),
            }
            # x = (hi*4 + lo2) * s_x, s_x folded into layer-1 weights
            x8 = res.tile([F, SHARD], dt.int8)
            nc.sync.dma_start(out=x8[:], in_=blob_d[0:F, 0:XC].bitcast(dt.int8))
            nc.vector.tensor_copy(out=fm["tx0"][:], in_=x8[:])
            nc.vector.tensor_scalar(out=fm["tx0"][:], in0=fm["tx0"][:],
                                    scalar1=4.0, scalar2=None,
                                    op0=mybir.AluOpType.mult)
            xl8 = res.tile([F, SHARD // 4], dt.uint8)
            nc.sync.dma_start(out=xl8[:], in_=blob_d[0:F, OFF_XL:OFF_XL + XLC].bitcast(dt.uint8))
            xli = res.tile([F, SHARD // 4], dt.int32)
            nc.vector.tensor_copy(out=xli[:], in_=xl8[:])
            x_lo = res.tile([F, SHARD], dt.bfloat16)
            x_lov = x_lo[:].rearrange("p (n four) -> p n four", four=4)
            nib = res.tile([F, SHARD // 4], dt.int32)
            for j in range(4):
                nc.vector.tensor_scalar(out=nib[:], in0=xli[:],
                                        scalar1=2 * j, scalar2=3,
                                        op0=mybir.AluOpType.logical_shift_right,
                                        op1=mybir.AluOpType.bitwise_and)
                nc.vector.tensor_copy(
                    out=x_lov[:, :, j:j + 1],
                    in_=nib[:].rearrange("p (n o) -> p n o", o=1))
            nc.vector.tensor_tensor(out=fm["tx0"][:], in0=fm["tx0"][:],
                                    in1=x_lo[:], op=mybir.AluOpType.add)

            idx_sb = res.tile([P, IDXCOLS], dt.int16)
            nc.sync.dma_start(out=idx_sb[0:16, 0:IDX1],
                              in_=blob_d[F:F + 16, 0:IDX1].bitcast(dt.int16))
            nc.sync.dma_start(out=idx_sb[0:16, IDX1:IDXCOLS],
                              in_=blob_d[F + 16:F + 32, 0:IDX2].bitcast(dt.int16))
            for g in range(1, 8):       # replicate idx into all 16-part groups
                nc.sync.dma_start(out=idx_sb[16 * g:16 * (g + 1), :],
                                  in_=idx_sb[0:16, :])

            w8 = res.tile([P, 2 * SPC], dt.uint8)
            nc.sync.dma_start(out=w8[:], in_=blob_d[:, OFF_W:OFF_W + SPC].bitcast(dt.uint8))
            w_edge = res.tile([P, ECOLS], dt.bfloat16)
            nc.vector.tensor_copy(out=w_edge[:], in_=w8[:, 0:ECOLS])
            # sqrt-coded: |w| = s_w * u^2 (scale folded host-side)
            nc.vector.tensor_tensor(out=w_edge[:], in0=w_edge[:], in1=w_edge[:],
                                    op=mybir.AluOpType.mult)
            sp8 = res.tile([P, 2 * SPC], dt.uint8)
            nc.sync.dma_start(out=sp8[:], in_=blob_d[:, OFF_SP:OFF_SP + SPC].bitcast(dt.uint8))

            # slot (bf16) and parity masks (bf16) from slot|par<<7
            slot_sb = res.tile([P, ECOLS], dt.bfloat16)
            par_sb = res.tile([P, ECOLS], dt.bfloat16)
            spi = res.tile([P, ECOLS], dt.int32)
            nc.vector.tensor_copy(out=spi[:], in_=sp8[:, 0:ECOLS])
            pari = res.tile([P, ECOLS], dt.int32)
            nc.vector.tensor_scalar(out=pari[:], in0=spi[:], scalar1=7, scalar2=None,
                                    op0=mybir.AluOpType.logical_shift_right)
            nc.vector.tensor_copy(out=par_sb[:], in_=pari[:])
            nc.vector.tensor_scalar(out=spi[:], in0=spi[:], scalar1=127, scalar2=None,
                                    op0=mybir.AluOpType.bitwise_and)
            nc.vector.tensor_copy(out=slot_sb[:], in_=spi[:])
            # per-candidate weights: even = w*(1-par), odd = w*par
            w_ev = res.tile([P, ECOLS], dt.bfloat16)
            w_od = res.tile([P, ECOLS], dt.bfloat16)
            nc.vector.tensor_tensor(out=w_od[:], in0=w_edge[:], in1=par_sb[:],
                                    op=mybir.AluOpType.mult)
            nc.vector.tensor_tensor(out=w_ev[:], in0=w_edge[:], in1=w_od[:],
                                    op=mybir.AluOpType.subtract)

            w_sb = res.tile([F, 6 * F + 2], dt.bfloat16)
            nc.sync.dma_start(out=w_sb[:], in_=blob_d[0:F, OFF_WM:OFF_WM + 6 * F + 2])
            bias_sb = res.tile([F, 2], dt.float32)
            nc.sync.dma_start(out=bias_sb[:],
                              in_=blob_d[0:F, OFF_B:OFF_B + 4].bitcast(dt.float32))
            blin_sb = res.tile([2, 1], dt.float32)
            nc.sync.dma_start(out=blin_sb[:],
                              in_=blob_d[0:2, OFF_B + 4:OFF_B + 6].bitcast(dt.float32))
            ident = res.tile([P, P], dt.bfloat16)
            make_identity(nc, ident[:])
            iota_i = res.tile([P, DTW], dt.int32)
            nc.gpsimd.iota(iota_i[:], pattern=[[1, DTW]], base=0, channel_multiplier=0)
            iota_b = res.tile([P, DTW], dt.bfloat16)
            nc.vector.tensor_copy(out=iota_b[:], in_=iota_i[:])

            # node-major staging for table writes
            s_nm = res.tile([P, DT * F], dt.bfloat16)

            # internal DRAM tables: [NP_PAD, 128] node-major, 128-col padded
            # rows; dma_gather views the same memory as [NPAIR, 256] pair rows
            bounce = [dram.tile([SHARD, PAIRW // 2], dt.bfloat16, name=f"bounce{i}")
                      for i in range(4 * nrep)]
            ag = [dram.tile([NP_PAD, PAIRW // 2], dt.bfloat16,
                            addr_space=("Local" if local_ag else "Shared"), name=f"ag{i}")
                  for i in range(4 * nrep)]

            # ---------- helpers ----------
            def fm_to_table(src_t, idx, tag):
                """PE-transpose fm shard into node-major staging, DMA to
                bounce[idx] (pair rows), allgather into ag[idx]."""
                with nc.named_scope(f"tw_{tag}"):
                    with tc.For_i(0, DT, 7) as j0:
                        for u in range(7):
                            stg = spool.tile([F, P], dt.bfloat16, tag="tstg")
                            nc.vector.tensor_copy(
                                out=stg[:], in_=src_t[0:F, ds((j0 + u) * P, P)])
                            pt = ptr.tile([P, F], dt.bfloat16, space="PSUM", tag="pt")
                            nc.tensor.transpose(out=pt[:], in_=stg[:],
                                                identity=ident[:F, :F])
                            nc.vector.tensor_copy(
                                out=s_nm[:, ds((j0 + u) * F, F)], in_=pt[:])
                    bo = bounce[idx]
                    nc.sync.dma_start(
                        out=bo[:].rearrange("(j p) f -> p j f", p=P)[:, :, 0:F],
                        in_=s_nm[:].rearrange("p (j f) -> p j f", f=F))
                    if local_ag:
                        for rr in range(C):
                            nc.sync.dma_start(
                                out=ag[idx][rr * SHARD:(rr + 1) * SHARD, :],
                                in_=bo[:])
                    else:
                        nc.gpsimd.collective_compute(
                            "AllGather", mybir.AluOpType.bypass,
                            replica_groups=rg, ins=[bo.opt()], outs=[ag[idx].opt()])

            GCALLS = ((1024, 8), (1024, 8), (128, 1))     # (idx per call, tiles)

            def prop_tile(table_ap, dest_fm, d_expr):
                """one dst-tile: 3 dma_gathers + one-hot build + 34 matmuls."""
                # stage idx/slot/w slices at static offsets (reg-offset APs are
                # not accepted by the custom DMA / PE weight port)
                stg_i = spool.tile([P, IDXT], dt.int16, tag="stgi")
                nc.vector.tensor_copy(out=stg_i[:], in_=idx_sb[:, ds(d_expr * IDXT, IDXT)])
                stg_s = spool.tile([P, 3 * TE2], dt.bfloat16, tag="stgs")
                nc.vector.tensor_copy(out=stg_s[:, 0:TE2],
                                      in_=slot_sb[:, ds(d_expr * TE2, TE2)])
                nc.vector.tensor_copy(out=stg_s[:, TE2:2 * TE2],
                                      in_=w_ev[:, ds(d_expr * TE2, TE2)])
                nc.vector.tensor_copy(out=stg_s[:, 2 * TE2:3 * TE2],
                                      in_=w_od[:, ds(d_expr * TE2, TE2)])

                ms = []
                icol = 0
                for nidx, ntile in GCALLS:
                    m_t = mpool.tile([P, ntile * PAIRW], dt.bfloat16, tag=f"m{ntile}")
                    nc.gpsimd.dma_gather(
                        out_ap=m_t[:].rearrange("p (t f) -> p t f", f=PAIRW),
                        in_ap=table_ap,
                        idxs_ap=stg_i[:, icol:icol + nidx // 16],
                        num_idxs=nidx,
                        num_idxs_reg=nidx,
                        elem_size=PAIRW,
                        single_packet=True,
                    )
                    icol += nidx // 16
                    ms.append((m_t, ntile))

                # weighted one-hot [128, TE2, 256]: cols 0:128 even cand,
                # 128:256 odd cand
                oh = opool.tile([P, TE2 * 2 * DTW], dt.bfloat16, tag="oh")
                ohv = oh[:].rearrange("p (t h s) -> p t h s", h=2, s=DTW)
                iota_v = iota_b[:].rearrange("p (o1 o2 s) -> p o1 o2 s", o1=1, o2=1) \
                                  .broadcast_to((P, TE2, 2, DTW))
                slot_v = stg_s[:, 0:TE2].rearrange("p (t o1 o2) -> p t o1 o2", o1=1, o2=1) \
                                        .broadcast_to((P, TE2, 2, DTW))
                nc.vector.tensor_tensor(out=ohv, in0=iota_v, in1=slot_v,
                                        op=mybir.AluOpType.is_equal)
                wev_v = stg_s[:, TE2:2 * TE2].rearrange("p (t o) -> p t o", o=1) \
                                             .broadcast_to((P, TE2, DTW))
                wod_v = stg_s[:, 2 * TE2:3 * TE2].rearrange("p (t o) -> p t o", o=1) \
                                                 .broadcast_to((P, TE2, DTW))
                nc.vector.tensor_tensor(out=ohv[:, :, 0, :], in0=ohv[:, :, 0, :],
                                        in1=wev_v, op=mybir.AluOpType.mult)
                nc.vector.tensor_tensor(out=ohv[:, :, 1, :], in0=ohv[:, :, 1, :],
                                        in1=wod_v, op=mybir.AluOpType.mult)

                ps = pscat.tile([F, DTW], dt.float32, space="PSUM", tag="ps")
                t = 0
                for m_t, ntile in ms:
                    for tt in range(ntile):
                        mv = m_t[:].rearrange("p (t f) -> p t f", f=PAIRW)
                        for h in range(2):
                            nc.tensor.matmul(
                                out=ps[:],
                                lhsT=mv[:, tt, h * P:h * P + F],
                                rhs=ohv[:, t, h, :],
                                start=(t == 0 and h == 0),
                                stop=(t == TE2 - 1 and h == 1),
                            )
                        t += 1
                nc.vector.tensor_copy(out=dest_fm[0:F, ds(d_expr * DTW, DTW)], in_=ps[:])

            def prop(table_t, dest_fm, tag):
                if "noprop" in cfg:
                    nc.vector.memset(dest_fm[:], 0)
                    return
                with nc.named_scope(f"prop_{tag}"):
                    # pair-row view [NPAIR, 256] of the node-major table
                    table_ap = table_t[:].rearrange("(n two) f -> n (two f)", two=2)
                    # gpsimd pre-touch executes the collective-completion wait
                    pr = spool.tile([1, 2], dt.bfloat16, tag="pr")
                    nc.gpsimd.dma_start(out=pr[:], in_=table_t[0:1, 0:2])
                    with tc.For_i(0, DT - 1, 2) as d0:
                        prop_tile(table_ap, dest_fm, d0)
                        prop_tile(table_ap, dest_fm, d0 + 1)
                    prop_tile(table_ap, dest_fm, DT - 1)

            def dense(layer, tx0_t, t1_t, s2_t, h_t):
                """h = relu(tx0@W0' + t1@W1 + s2@W2') feature-major, bf16."""
                with nc.named_scope(f"dense_{layer}"):
                    wof = layer * 3 * F

                    def chunk(c0, width):
                        pd = pdense.tile([F, NCHUNK], dt.float32, space="PSUM", tag="pd")
                        for ki, rhs_t in enumerate((tx0_t, t1_t, s2_t)):
                            nc.tensor.matmul(
                                out=pd[:, :width],
                                lhsT=w_sb[:, wof + ki * F:wof + (ki + 1) * F],
                                rhs=rhs_t[0:F, ds(c0, width)],
                                start=(ki == 0),
                                stop=(ki == 2),
                            )
                        nc.scalar.activation(
                            out=h_t[0:F, ds(c0, width)], in_=pd[:, :width],
                            func=mybir.ActivationFunctionType.Relu,
                            bias=bias_sb[:, layer:layer + 1],
                        )

                    nfull = SHARD // NCHUNK
                    with tc.For_i(0, nfull * NCHUNK, 2 * NCHUNK) as c0:
                        chunk(c0, NCHUNK)
                        chunk(c0 + NCHUNK, NCHUNK)
                    chunk(nfull * NCHUNK, SHARD - nfull * NCHUNK)

            # ---------- pipeline ----------
            for _r in range(nrep):
                rb = 4 * _r
                fm_to_table(fm["tx0"], rb + 3, "x")       # ag[3] = x table

                prop(ag[rb + 3], fm["t1"], "l1a")         # t1 = L @ x
                fm_to_table(fm["t1"], rb + 0, "t1")
                prop(ag[rb + 0], fm["s2"], "l1b")         # s2 = L @ t1
                dense(0, fm["tx0"], fm["t1"], fm["s2"], fm["h"])
                fm_to_table(fm["h"], rb + 1, "h1")

                prop(ag[rb + 1], fm["t1"], "l2a")
                fm_to_table(fm["t1"], rb + 2, "t1b")
                prop(ag[rb + 2], fm["s2"], "l2b")
                dense(1, fm["h"], fm["t1"], fm["s2"],
                      fm["tx0" if nrep == 1 else "h"])

            hfin = fm["tx0" if nrep == 1 else "h"]
            with nc.named_scope("final"):
                nchunks = (SHARD + NCHUNK - 1) // NCHUNK
                for ci in range(nchunks):
                    c0 = ci * NCHUNK
                    c1 = min(SHARD, c0 + NCHUNK)
                    pf = pdense.tile([2, NCHUNK], dt.float32, space="PSUM", tag="pd")
                    nc.tensor.matmul(out=pf[:, :c1 - c0],
                                     lhsT=w_sb[:, 6 * F:6 * F + 2],
                                     rhs=hfin[0:F, c0:c1], start=True, stop=True)
                    ot = spool.tile([2, NCHUNK], dt.bfloat16, tag="ot")
                    nc.scalar.activation(
                        out=ot[:, :c1 - c0], in_=pf[:, :c1 - c0],
                        func=mybir.ActivationFunctionType.Identity,
                        bias=blin_sb[:],
                    )
                    nc.sync.dma_start(out=out_d[:, c0:c1], in_=ot[:, :c1 - c0])

    nc.compile()
    # memoize the BIR json: bass2jax re-serializes it on every call otherwise
    _json = nc.to_json_bytes()
    nc.to_json_bytes = lambda: _json
    return nc


# --------------------------------------------------------------------------
# input packing
# --------------------------------------------------------------------------
def _pack_inputs(x, edge_index, edge_weight, W1, b1, W2, b2, Wlin, blin):
    new_id, idxp, wp, sp = _preprocess_cached(x, edge_index, edge_weight)

    # 10-bit x (scale s_x) and sqrt-coded u8 |norm_w|: the device computes
    # G = sum_e u_e^2 * (.), so L_hat = sigma_w * G with sigma_w = -M/255^2;
    # all scale corrections fold into the dense weights.
    s_x = float(np.abs(x).max()) / 511.0 if np.abs(x).max() > 0 else 1.0
    x_q = np.clip(np.round(x / s_x), -511, 511).astype(np.int16)
    x_hi = (x_q >> 2).astype(np.int8)
    x_lo = (x_q & 3).astype(np.uint8)
    M = float(np.abs(wp).max())
    sgw = -(M if M > 0 else 1.0) / (255.0 * 255.0)
    w_u8 = np.clip(np.round(np.sqrt(np.abs(wp) / (M if M > 0 else 1.0)) * 255.0),
                   0, 255).astype(np.uint8)

    hp = np.zeros((NP_PAD, F), np.int8)
    hp[new_id] = x_hi
    hpT = hp.T
    lp = np.zeros((NP_PAD, F), np.uint8)
    lp[new_id] = x_lo
    lpT = lp.T

    wall = np.concatenate([
        s_x * (W1[0] - W1[2]), s_x * sgw * W1[1], s_x * sgw * sgw * 2.0 * W1[2],
        W2[0] - W2[2], sgw * W2[1], sgw * sgw * 2.0 * W2[2],
    ], axis=1).astype(BF)
    wall = np.concatenate([wall, Wlin.astype(BF)], axis=1)

    # idx wrapped for dma_gather: per dst-tile block of IDXT cols; idx j of the
    # tile's 2176-list sits at (j%16, block + j//16)
    in_maps = []
    for c in range(C):
        idxw = idxp[c].reshape(P, DT, TE2).transpose(1, 2, 0).reshape(DT, IDXT, 16)
        idxw = idxw.transpose(0, 2, 1).reshape(DT * 16, IDXT)  # [(D,16p), IDXT]
        idx_plane = np.zeros((16, IDXCOLS), np.int16)
        for d in range(DT):
            idx_plane[:, d * IDXT:(d + 1) * IDXT] = idxw[d * 16:(d + 1) * 16, :]

        blob = np.zeros((P, CB), BF)
        xs = np.ascontiguousarray(hpT[:, c * SHARD:(c + 1) * SHARD])
        blob[0:F, 0:XC] = xs.view(np.int16).view(BF)
        ls = np.ascontiguousarray(lpT[:, c * SHARD:(c + 1) * SHARD])
        ls4 = ls.reshape(F, SHARD // 4, 4)
        lpack = (ls4[:, :, 0] | (ls4[:, :, 1] << 2) | (ls4[:, :, 2] << 4)
                 | (ls4[:, :, 3] << 6)).astype(np.uint8)
        blob[0:F, OFF_XL:OFF_XL + XLC] = lpack.view(np.uint16).view(BF)
        blob[F:F + 16, 0:IDX1] = idx_plane[:, 0:IDX1].view(BF)
        blob[F + 16:F + 32, 0:IDX2] = idx_plane[:, IDX1:IDXCOLS].view(BF)
        wu = np.zeros((P, SPC * 2), np.uint8)
        wu[:, 0:ECOLS] = w_u8[c]
        blob[:, OFF_W:OFF_W + SPC] = wu.view(np.uint16).view(BF)
        spc = np.zeros((P, SPC * 2), np.uint8)
        spc[:, 0:ECOLS] = sp[c]
        blob[:, OFF_SP:OFF_SP + SPC] = spc.view(np.uint16).view(BF)
        blob[0:F, OFF_WM:OFF_WM + 6 * F + 2] = wall
        blob[0:F, OFF_B:OFF_B + 2] = b1.astype(np.float32).view(np.uint16).view(BF).reshape(F, 2)
        blob[0:F, OFF_B + 2:OFF_B + 4] = b2.astype(np.float32).view(np.uint16).view(BF).reshape(F, 2)
        blob[0:2, OFF_B + 4:OFF_B + 6] = blin.astype(np.float32).view(np.uint16).view(BF).reshape(2, 2)
        in_maps.append({"blob": blob})
    return new_id, in_maps


# --------------------------------------------------------------------------
# entry point
# --------------------------------------------------------------------------
def kernel(x, edge_index, edge_weight, W1, b1, W2, b2, Wlin, blin,
           _trace=False, _tmpdir=None):
    global _compiled
    x = np.asarray(x, np.float32)
    W1 = np.asarray(W1, np.float32); W2 = np.asarray(W2, np.float32)
    b1 = np.asarray(b1, np.float32); b2 = np.asarray(b2, np.float32)
    Wlin = np.asarray(Wlin, np.float32); blin = np.asarray(blin, np.float32)

    new_id, in_maps = _pack_inputs(x, edge_index, edge_weight,
                                   W1, b1, W2, b2, Wlin, blin)

    if _compiled is None:
        _compiled = _build_kernel()
    nc = _compiled

    import time as _time
    _t0 = _time.perf_counter()
    try:
        res = run_bass_kernel_spmd(nc, in_maps, core_ids=list(range(C)),
                                   trace=_trace, tmpdir=_tmpdir)
    except ModuleNotFoundError:
        res = run_bass_kernel_spmd(nc, in_maps, core_ids=list(range(C)),
                                   trace=False, tmpdir=_tmpdir)
    kernel.last_spmd_wall_s = _time.perf_counter() - _t0

    outs_per_core = [np.asarray(res.results[c]["out"]) for c in range(len(res.results))]
    out_p = np.concatenate(outs_per_core, axis=1)   # [2, NP_PAD]
    out = out_p.T[new_id].astype(np.float32)
    if _trace:
        kernel.last_exec_time_ns = res.exec_time_ns
        kernel.last_results = res
    return out
